# revision 19
# baseline (speedup 1.0000x reference)
"""Trainium2 Bass kernel: Performer (linear) attention + in/out projections.

Problem nn_LinearPerformerAttention_6717328851263:
  x:(4,4096,1024) f32, w_qkv:(1024,3072), proj_matrix:(16,64,256),
  w_out:(1024,1024), b_out:(1024,)

Sharding over 8 cores: core c -> (batch b=c//2, head-group g=c%2: 8 of 16 heads).
Host gather: out[b] = y_(b,0) + y_(b,1) + b_out.

v3: fp16 matmul data (PSUM fp32), software-pipelined so the Tensor engine
streams without dependency gaps (gaps reset the PE p-state ramp: full
2.4 GHz only after ~3us of continuous execution):
  pass A group g emits: qT/kT/v projections (g), k_proj matmuls (g),
    kv-state matmuls for group g-1 (whose elu outputs are long since done).
  pass B group g emits: q_proj matmuls (g), y matmuls for group g-1,
    attn matmuls (g) -- every consumer is >=3us of PE work behind its
    producer's elementwise chain.
elu1(x) = elu(x)+1 = min(exp(x), 1+relu(x))  [identity: for x>0 exp(x)>=1+x]
  E = exp(x): Scalar (PSUM->fp16); R' = (x max 0)+1: one DVE tensor_scalar
  (PSUM 1x); combine = TT min(E,R'): fp16 SBUF 2x mode on DVE, or GpSimd
  (no PSUM port, but SBUF fp16 ok) -- split by a knob to balance engines.
kv state via flipped matmuls: lhsT=k_proj slab [tok,Fchunk], rhs=[v|1]
  [tok,65] -> [Fchunk, 64+1] directly in the pass-B lhsT layout (col 64 =
  k_sum free); no transposes.  attn pair-packed as in baseline (kvS zero-
  padded, ksr k_sum-replicated) so z normalization is two [128,512] DVE ops.
All DMA issues ride the GpSimd queue.
"""

import numpy as np
from contextlib import ExitStack

import concourse.bass as bass
import concourse.bacc as bacc
import concourse.tile as tile
from concourse import mybir
from concourse.bass_utils import run_bass_kernel_spmd

FP32 = mybir.dt.float32
F16 = mybir.dt.float16
AL = mybir.AluOpType
AF = mybir.ActivationFunctionType

B, SEQ, D = 4, 4096, 1024
H, HD, F = 16, 64, 256
HPC = 8            # heads per core
DH = HPC * HD      # 512 head-space dims per core
P = 128
NCORES = 8


def _emit(tc, n, xT, wq, wk, wv, proj, wout, y, qTd):
    nc = tc.nc
    NG = n // 512       # token groups
    TPG = 4             # 128-token tiles per group

    ctx = ExitStack()
    with ctx:
        const = ctx.enter_context(tc.tile_pool(name="const", bufs=1))

        # ---- startup: issue wq + first x tile first so qT matmuls can
        # start while everything else loads.
        wpool = ctx.enter_context(tc.tile_pool(name="wpool", bufs=1))
        wq_sb = [wpool.tile([P, DH], F16, tag=f"wq{s}", name=f"wq{s}") for s in range(8)]
        for s in range(8):
            nc.gpsimd.dma_start(out=wq_sb[s], in_=wq[s * P:(s + 1) * P, :])

        xT_v = xT.rearrange("(s p) m -> p s m", p=P)
        xtpool = ctx.enter_context(tc.tile_pool(name="xtpool", bufs=2))
        xt0 = xtpool.tile([P, 8, 512], F16, tag="xt", name="xt")
        nc.gpsimd.dma_start(out=xt0, in_=xT_v[:, :, 0:512])

        wk_sb = [wpool.tile([P, DH], F16, tag=f"wk{s}", name=f"wk{s}") for s in range(8)]
        wv_sb = [wpool.tile([P, DH], F16, tag=f"wv{s}", name=f"wv{s}") for s in range(8)]
        for s in range(8):
            nc.sync.dma_start(out=wk_sb[s], in_=wk[s * P:(s + 1) * P, :])
            nc.scalar.dma_start(out=wv_sb[s], in_=wv[s * P:(s + 1) * P, :])
        proj_pair = [const.tile([P, F], F16, tag=f"projp{i}", name=f"projp{i}")
                     for i in range(4)]
        for i in range(4):
            nc.sync.dma_start(out=proj_pair[i], in_=proj[i * P:(i + 1) * P, :])

        ones_sb = const.tile([P, P], F16, tag="ones_sb", name="ones_sb")
        nc.vector.memset(ones_sb, 1.0)

        # q_proj elu outputs for the first NQA groups, computed in pass A
        # (pass B is elementwise-bound; pass A is PE-bound -- shift work)
        NQA = 3
        qpa_pool = ctx.enter_context(tc.tile_pool(name="qpa", bufs=NQA))
        qPsaved = {}

        kvS = [[const.tile([P, P], F16, tag=f"kvS{h}_{s}", name=f"kvS{h}_{s}")
                for s in range(2)] for h in range(HPC)]
        ksr = [[const.tile([P, P], F16, tag=f"ksr{h}_{s}", name=f"ksr{h}_{s}")
                for s in range(2)] for h in range(HPC)]

        # ---------------- pass A ----------------
        with ExitStack() as actx:
            kvaccp = actx.enter_context(tc.tile_pool(name="kvaccp", bufs=1))
            kv_acc = [kvaccp.tile([P, 260], FP32, tag=f"kva{i}", name=f"kva{i}")
                      for i in range(4)]
            ktpool = actx.enter_context(tc.tile_pool(name="ktpool", bufs=2))
            qtpool = actx.enter_context(tc.tile_pool(name="qtpool", bufs=3))
            vpool = actx.enter_context(tc.tile_pool(name="vpool", bufs=2))
            ketmp = actx.enter_context(tc.tile_pool(name="ketmp", bufs=3))
            kppool = actx.enter_context(tc.tile_pool(name="kppool", bufs=2))
            mmps = actx.enter_context(tc.tile_pool(name="mmps", bufs=2, space="PSUM"))
            kpps = actx.enter_context(tc.tile_pool(name="kpps", bufs=3, space="PSUM"))

            prev = None  # (kP tiles dict, vone) of group g-1

            def emit_kv_half(kv_ps, hp, j, pkP, pvone):
                """kv-state matmuls (head j of pair hp) of a PREVIOUS group.
                pkP[tp] cols: j*512 + ti*256 + f."""
                for s in range(2):
                    co = j * 130 + s * 65
                    for t in range(TPG):
                        base = j * 512 + (t % 2) * F + s * P
                        nc.tensor.matmul(
                            kv_ps[:, co:co + 65],
                            lhsT=(pkP[t // 2][:, base:base + P]),
                            rhs=(pvone[:, t, 2 * hp + j, :]),
                            start=(t == 0), stop=(t == TPG - 1),
                            skip_group_check=True)

            def fold_kv(hp, kv_ps, g):
                if g == 0:
                    nc.vector.tensor_copy(kv_acc[hp], kv_ps)
                else:
                    nc.vector.tensor_tensor(
                        out=kv_acc[hp], in0=kv_ps, in1=kv_acc[hp], op=AL.add)

            for g in range(NG):
                g0 = g * 512
                if g == 0:
                    xt = xt0
                else:
                    xt = xtpool.tile([P, 8, 512], F16, tag="xt", name="xt")
                    nc.gpsimd.dma_start(out=xt, in_=xT_v[:, :, g0:g0 + 512])

                # qT: spill to DRAM scratch (fp16)
                qt_g = []
                for fs in range(4):
                    ps = mmps.tile([P, 512], FP32, tag="mm", name="mm")
                    for s in range(8):
                        nc.tensor.matmul(
                            ps, lhsT=(wq_sb[s][:, fs * P:(fs + 1) * P]),
                            rhs=(xt[:, s, :]), start=(s == 0), stop=(s == 7))
                    qt_sb = qtpool.tile([P, 512], F16, tag="qt", name="qt")
                    nc.scalar.copy(qt_sb, ps)
                    qt_g.append(qt_sb)
                    if g >= NQA:
                        nc.gpsimd.dma_start(
                            out=qTd[fs * P:(fs + 1) * P, g0:g0 + 512], in_=qt_sb)

                # kT: kept in SBUF for this group
                kt_sb = [ktpool.tile([P, 512], F16, tag=f"kt{fs}", name=f"kt{fs}")
                         for fs in range(4)]
                for fs in range(4):
                    ps = mmps.tile([P, 512], FP32, tag="mm", name="mm")
                    for s in range(8):
                        nc.tensor.matmul(
                            ps, lhsT=(wk_sb[s][:, fs * P:(fs + 1) * P]),
                            rhs=(xt[:, s, :]), start=(s == 0), stop=(s == 7))
                    nc.scalar.copy(kt_sb[fs], ps)

                # v with ones column: vone[p, t, h, 0:64]=v, [..,64]=1
                # (pool depth 2 -> generation g reuses g-2's buffer, whose
                # ones column is already set; write it only for g<2)
                vone = vpool.tile([P, TPG, HPC, HD + 1], F16, tag="vone", name="vone")
                if g < 2:
                    nc.scalar.copy(
                        vone[:, :, :, HD],
                        ones_sb[:, 0:TPG * HPC].rearrange(
                            "p (t h) -> p t h", t=TPG))
                for t in range(TPG):
                    ps = mmps.tile([P, 512], FP32, tag="mm", name="mm")
                    for s in range(8):
                        nc.tensor.matmul(
                            ps, lhsT=(xt[:, s, t * P:(t + 1) * P]),
                            rhs=(wv_sb[s]), start=(s == 0), stop=(s == 7))
                    nc.vector.tensor_copy(
                        vone[:, t, :, 0:HD],
                        ps.rearrange("p (h e) -> p h e", h=HPC))

                # k_proj + elu1 for THIS group; kv matmuls for the PREVIOUS
                # group interleaved per pair to keep the PE streaming while
                # this group's elu chain runs on Scalar/DVE/GpSimd.
                kP_g = []
                for hp in range(HPC // 2):
                    # kp: [128,1024] 2-bank tile, cols j*512 + ti*256 -> one
                    # exp / one tensor_scalar / one TT-min per (hp,tp)
                    kP = [kppool.tile([P, 1024], F16, tag=f"kP{hp}_{tp}",
                                      name=f"kP{hp}_{tp}") for tp in range(2)]
                    kv_ps_full = mmps.tile([P, 512], FP32, tag="mm", name="kvg") \
                        if prev is not None else None
                    kv_ps = kv_ps_full[:, 0:260] if prev is not None else None
                    for tp in range(2):
                        kp = kpps.tile([P, 1024], FP32, tag="kp", name="kp")
                        for ti in range(2):
                            t = tp * 2 + ti
                            for j in range(2):
                                hb = j * HD
                                nc.tensor.matmul(
                                    kp[:, j * 512 + ti * F:
                                        j * 512 + (ti + 1) * F],
                                    lhsT=(kt_sb[hp][hb:hb + HD,
                                                    t * P:(t + 1) * P]),
                                    rhs=(proj_pair[hp][hb:hb + HD, :]),
                                    start=True, stop=True,
                                    skip_group_check=True)
                        kE = ketmp.tile([P, 1024], F16, tag="kE", name="kE")
                        kR = ketmp.tile([P, 1024], F16, tag="kR", name="kR")
                        nc.scalar.activation(kE, kp, AF.Exp)
                        nc.vector.tensor_scalar(
                            kR, kp, 0.0, 1.0, op0=AL.max, op1=AL.add)
                        nc.vector.tensor_tensor(
                            out=kP[tp], in0=kE, in1=kR, op=AL.min)
                        # kv matmuls of the PREVIOUS group for head j=tp,
                        # between this group's kproj bursts: spreads PSUM
                        # bank reuse and keeps the PE streaming.
                        if kv_ps is not None:
                            emit_kv_half(kv_ps, hp, tp, prev[0][hp], prev[1])
                    kP_g.append(kP)
                    if kv_ps is not None:
                        fold_kv(hp, kv_ps, g - 1)
                    if g < NQA:
                        # q_proj + elu1 for this (early) group's pair, using
                        # the un-spilled qT tiles; results held in SBUF for
                        # pass B.  PSUM rides the kproj pool rotation.
                        qP2 = [qpa_pool.tile([P, 1024], F16, tag=f"qPa{hp}_{j}",
                                             name=f"qPa{hp}_{j}")
                               for j in range(2)]
                        qps2 = [kpps.tile([P, 1024], FP32, tag="kp",
                                          name="qpa_ps") for j in range(2)]
                        for s in range(2):
                            for j in range(2):
                                hb = j * HD
                                nc.tensor.matmul(
                                    qps2[j][:, s * 512:(s + 1) * 512],
                                    lhsT=(proj_pair[hp][hb:hb + HD,
                                                        s * P:(s + 1) * P]),
                                    rhs=(qt_g[hp][hb:hb + HD, :]),
                                    start=True, stop=True,
                                    skip_group_check=True)
                        for j in range(2):
                            qE2 = ketmp.tile([P, 1024], F16, tag="kE", name="qE2")
                            qR2 = ketmp.tile([P, 1024], F16, tag="kR", name="qR2")
                            nc.scalar.activation(qE2, qps2[j], AF.Exp)
                            nc.vector.tensor_scalar(
                                qR2, qps2[j], 0.0, 1.0, op0=AL.max, op1=AL.add)
                            nc.vector.tensor_tensor(
                                out=qP2[j], in0=qE2, in1=qR2, op=AL.min)
                        qPsaved[(g, hp)] = qP2
                prev = (kP_g, vone)

            for hp in range(HPC // 2):
                kv_ps_full = mmps.tile([P, 512], FP32, tag="mm", name="kvg")
                kv_ps = kv_ps_full[:, 0:260]
                for j in range(2):
                    emit_kv_half(kv_ps, hp, j, prev[0][hp], prev[1])
                fold_kv(hp, kv_ps, NG - 1)

            # zero the attn lhsT pads late so the memsets don't clog the
            # DVE queue ahead of group-0's PSUM readers
            for h in range(HPC):
                for s in range(2):
                    nc.vector.memset(kvS[h][s], 0.0)
                    nc.vector.memset(ksr[h][s], 0.0)

            # ------- kv fixup: kv_acc -> kvS (cast, GpSimd) / ksr (Scalar) ----
            for hp in range(4):
                for j in range(2):
                    h = 2 * hp + j
                    hb = j * HD
                    for s in range(2):
                        co = j * 130 + s * 65
                        nc.vector.tensor_copy(
                            kvS[h][s][:, hb:hb + HD],
                            kv_acc[hp][:, co:co + HD])
                        nc.scalar.activation(
                            ksr[h][s][:, hb:hb + HD], ones_sb[:, 0:HD],
                            AF.Copy, scale=kv_acc[hp][:, co + HD:co + HD + 1])

        # ---------------- pass B ----------------
        with ExitStack() as bctx:
            wopool = bctx.enter_context(tc.tile_pool(name="wopool", bufs=1))
            wo_sb = [wopool.tile([P, D], F16, tag=f"wo{s}", name=f"wo{s}") for s in range(4)]
            for s in range(4):
                nc.gpsimd.dma_start(out=wo_sb[s], in_=wout[s * P:(s + 1) * P, :])

            qtbpool = bctx.enter_context(tc.tile_pool(name="qtbpool", bufs=2))
            qepool = bctx.enter_context(tc.tile_pool(name="qepool", bufs=3))
            qppool = bctx.enter_context(tc.tile_pool(name="qppool", bufs=2))
            attpool = bctx.enter_context(tc.tile_pool(name="attpool", bufs=2))
            zpool = bctx.enter_context(tc.tile_pool(name="zpool", bufs=2))
            ypool = bctx.enter_context(tc.tile_pool(name="ypool", bufs=3))
            qpps = bctx.enter_context(tc.tile_pool(name="qpps", bufs=2, space="PSUM"))
            atps = bctx.enter_context(tc.tile_pool(name="atps", bufs=2, space="PSUM"))
            dnps = bctx.enter_context(tc.tile_pool(name="dnps", bufs=1, space="PSUM"))
            yps = bctx.enter_context(tc.tile_pool(name="yps", bufs=1, space="PSUM"))

            qTd_v = qTd.rearrange("(hh p) m -> p hh m", p=P)

            att_prev = None
            g0_prev = 0

            def emit_y_half(t, o, patt, pg0):
                """y matmuls for token tile t, output half o, of a
                PREVIOUS group.  The eviction is emitted before the current
                slot's exps so it sits ahead of them in the scalar queue."""
                pso = yps.tile([P, 512], FP32, tag="y", name="pso")
                for s in range(4):
                    nc.tensor.matmul(
                        pso, lhsT=(patt[s][:, t * P:(t + 1) * P]),
                        rhs=(wo_sb[s][:, o * 512:(o + 1) * 512]),
                        start=(s == 0), stop=(s == 3))
                y_sb = ypool.tile([P, 512], F16, tag="ysb", name="ysb")
                nc.scalar.copy(y_sb, pso)
                nc.gpsimd.dma_start(
                    out=y[pg0 + t * P: pg0 + (t + 1) * P,
                          o * 512:(o + 1) * 512],
                    in_=y_sb)

            for g in range(NG):
                g0 = g * 512
                early = g < NQA
                if not early:
                    qt = qtbpool.tile([P, HPC // 2, 512], F16, tag="qt", name="qt")
                    nc.gpsimd.dma_start(out=qt, in_=qTd_v[:, :, g0:g0 + 512])

                # q_proj matmuls + elu1 for this group (early groups: done in
                # pass A), y matmuls for the previous group interleaved.
                qP_g = []
                for hp in range(HPC // 2):
                    if early:
                        qP_g.append(qPsaved[(g, hp)])
                        if att_prev is not None:
                            emit_y_half(hp, 0, att_prev, g0_prev)
                            emit_y_half(hp, 1, att_prev, g0_prev)
                        continue
                    # qp[j]: [128,1024] 2-bank tile, cols s*512 -> one exp /
                    # one tensor_scalar / one TT-min per (hp, j)
                    qP = [qppool.tile([P, 1024], F16, tag=f"qP{hp}_{j}",
                                      name=f"qP{hp}_{j}") for j in range(2)]
                    qps = [qpps.tile([P, 1024], FP32, tag="qp", name="qp")
                           for j in range(2)]
                    for s in range(2):
                        for j in range(2):
                            hb = j * HD
                            nc.tensor.matmul(
                                qps[j][:, s * 512:(s + 1) * 512],
                                lhsT=(proj_pair[hp][hb:hb + HD,
                                                    s * P:(s + 1) * P]),
                                rhs=(qt[hb:hb + HD, hp, :]),
                                start=True, stop=True,
                                skip_group_check=True)
                        if att_prev is not None:
                            emit_y_half(hp, s, att_prev, g0_prev)
                    for j in range(2):
                        qE = qepool.tile([P, 1024], F16, tag="qE", name="qE")
                        qR = qepool.tile([P, 1024], F16, tag="qR", name="qR")
                        nc.scalar.activation(qE, qps[j], AF.Exp)
                        nc.vector.tensor_scalar(
                            qR, qps[j], 0.0, 1.0, op0=AL.max, op1=AL.add)
                        nc.vector.tensor_tensor(
                            out=qP[j], in0=qE, in1=qR, op=AL.min)
                    qP_g.append(qP)

                att_sb = [attpool.tile([P, 512], F16, tag=f"att{i}", name=f"att{i}")
                          for i in range(4)]
                for hp in range(HPC // 2):
                    aps = atps.tile([P, 512], FP32, tag="at", name="aps")
                    dps = dnps.tile([P, 512], FP32, tag="dn", name="dps")
                    for j in range(2):
                        h = 2 * hp + j
                        for s in range(2):
                            first = j == 0 and s == 0
                            last = j == 1 and s == 1
                            rhs = qP_g[hp][j][:, s * 512:(s + 1) * 512]
                            nc.tensor.matmul(
                                aps, lhsT=(kvS[h][s]), rhs=rhs,
                                start=first, stop=last, skip_group_check=True)
                            nc.tensor.matmul(
                                dps, lhsT=(ksr[h][s]), rhs=rhs,
                                start=first, stop=last, skip_group_check=True)
                    zb = zpool.tile([P, 512], FP32, tag="zb", name="zb")
                    nc.vector.reciprocal_approx_fast(zb, dps)
                    nc.vector.tensor_tensor(
                        out=att_sb[hp], in0=aps, in1=zb, op=AL.mult)

                att_prev = att_sb
                g0_prev = g0

            for t in range(TPG):
                for o in range(2):
                    emit_y_half(t, o, att_prev, g0_prev)


def build(n=SEQ):
    nc = bacc.Bacc("TRN2", target_bir_lowering=False, debug=False,
                   enable_asserts=False)
    xT = nc.declare_dram_parameter("xT", [D, n], F16, isOutput=False)
    wq = nc.declare_dram_parameter("wq", [D, DH], F16, isOutput=False)
    wk = nc.declare_dram_parameter("wk", [D, DH], F16, isOutput=False)
    wv = nc.declare_dram_parameter("wv", [D, DH], F16, isOutput=False)
    proj = nc.declare_dram_parameter("proj", [DH, F], F16, isOutput=False)
    wout = nc.declare_dram_parameter("wout", [DH, D], F16, isOutput=False)
    y = nc.declare_dram_parameter("y", [n, D], F16, isOutput=True)
    qTd = nc.dram_tensor("qT_scratch", [DH, n], F16)
    with tile.TileContext(nc) as tc:
        _emit(tc, n, xT, wq, wk, wv, proj, wout, y, qTd)
    nc.finalize()
    return nc


def make_in_maps(x, w_qkv, proj_matrix, w_out):
    x = np.asarray(x, np.float32)
    w_qkv = np.asarray(w_qkv, np.float32)
    proj_matrix = np.asarray(proj_matrix, np.float32)
    w_out = np.asarray(w_out, np.float32)
    in_maps = []
    for c in range(NCORES):
        b, g = c // 2, c % 2
        in_maps.append({
            "xT": x[b].T.astype(np.float16),
            "wq": w_qkv[:, DH * g:DH * (g + 1)].astype(np.float16),
            "wk": w_qkv[:, D + DH * g:D + DH * (g + 1)].astype(np.float16),
            "wv": w_qkv[:, 2 * D + DH * g:2 * D + DH * (g + 1)].astype(np.float16),
            "proj": proj_matrix[HPC * g:HPC * (g + 1)].reshape(DH, F)
                    .astype(np.float16),
            "wout": w_out[DH * g:DH * (g + 1), :].astype(np.float16),
        })
    return in_maps


_NC_CACHE = {}


def get_nc(n=SEQ):
    if n not in _NC_CACHE:
        _NC_CACHE[n] = build(n)
    return _NC_CACHE[n]


def _install_ntff_hook_shim():
    """The agent image's antenv lacks axon_hooks; recreate it so
    run_bass_kernel_spmd(trace=True) can capture NTFF profiles."""
    import sys
    import types
    try:
        from antenv.axon_hooks import get_axon_ntff_profile_hook  # noqa: F401
        return True
    except ImportError:
        pass
    try:
        from trn_agent_boot.trn_boot import _ntff_profile_via_ctypes
        import antenv
        mod = types.ModuleType("antenv.axon_hooks")
        mod._hook = _ntff_profile_via_ctypes("/opt/axon/libaxon_pjrt.so")
        mod.set_axon_ntff_profile_hook = lambda h: setattr(mod, "_hook", h)
        mod.get_axon_ntff_profile_hook = lambda: mod._hook
        sys.modules["antenv.axon_hooks"] = mod
        antenv.axon_hooks = mod
        return True
    except Exception as e:  # profiling is best-effort
        print(f"ntff hook shim failed: {e}")
        return False


def run(x, w_qkv, proj_matrix, w_out, b_out, trace=False, **kw):
    if trace:
        _install_ntff_hook_shim()
    nc = get_nc(SEQ)
    in_maps = make_in_maps(x, w_qkv, proj_matrix, w_out)
    res = run_bass_kernel_spmd(nc, in_maps, list(range(NCORES)),
                               trace=trace, **kw)
    b_out = np.asarray(b_out, np.float32)
    out = np.empty((B, SEQ, D), np.float32)
    for b in range(B):
        out[b] = res.results[2 * b]["y"].astype(np.float32) \
            + res.results[2 * b + 1]["y"].astype(np.float32) \
            + b_out[None, :]
    return out, res


def kernel(x, w_qkv, proj_matrix, w_out, b_out):
    out, _ = run(x, w_qkv, proj_matrix, w_out, b_out)
    return out


# revision 20
# speedup vs baseline: 1.0921x; 1.0921x over previous
"""Trainium2 Bass kernel: Performer (linear) attention + in/out projections.

Problem nn_LinearPerformerAttention_6717328851263:
  x:(4,4096,1024) f32, w_qkv:(1024,3072), proj_matrix:(16,64,256),
  w_out:(1024,1024), b_out:(1024,)

Sharding over 8 cores: core c -> (batch b=c//2, head-group g=c%2: 8 of 16 heads).
Host gather: out[b] = y_(b,0) + y_(b,1) + b_out.

v3: fp16 matmul data (PSUM fp32), software-pipelined so the Tensor engine
streams without dependency gaps (gaps reset the PE p-state ramp: full
2.4 GHz only after ~3us of continuous execution):
  pass A group g emits: qT/kT/v projections (g), k_proj matmuls (g),
    kv-state matmuls for group g-1 (whose elu outputs are long since done).
  pass B group g emits: q_proj matmuls (g), y matmuls for group g-1,
    attn matmuls (g) -- every consumer is >=3us of PE work behind its
    producer's elementwise chain.
elu1(x) = elu(x)+1 = min(exp(x), 1+relu(x))  [identity: for x>0 exp(x)>=1+x]
  E = exp(x): Scalar (PSUM->fp16); R' = (x max 0)+1: one DVE tensor_scalar
  (PSUM 1x); combine = TT min(E,R'): fp16 SBUF 2x mode on DVE, or GpSimd
  (no PSUM port, but SBUF fp16 ok) -- split by a knob to balance engines.
kv state via flipped matmuls: lhsT=k_proj slab [tok,Fchunk], rhs=[v|1]
  [tok,65] -> [Fchunk, 64+1] directly in the pass-B lhsT layout (col 64 =
  k_sum free); no transposes.  attn pair-packed as in baseline (kvS zero-
  padded, ksr k_sum-replicated) so z normalization is two [128,512] DVE ops.
All DMA issues ride the GpSimd queue.
"""

import numpy as np
from contextlib import ExitStack

import concourse.bass as bass
import concourse.bacc as bacc
import concourse.tile as tile
from concourse import mybir
from concourse.bass_utils import run_bass_kernel_spmd

FP32 = mybir.dt.float32
F16 = mybir.dt.float16
AL = mybir.AluOpType
AF = mybir.ActivationFunctionType

B, SEQ, D = 4, 4096, 1024
H, HD, F = 16, 64, 256
HPC = 8            # heads per core
DH = HPC * HD      # 512 head-space dims per core
P = 128
NCORES = 8


def _emit(tc, n, xT, wq, wk, wv, proj, wout, y, qTd):
    nc = tc.nc
    NG = n // 512       # token groups
    TPG = 4             # 128-token tiles per group

    ctx = ExitStack()
    with ctx:
        const = ctx.enter_context(tc.tile_pool(name="const", bufs=1))

        # ---- startup: issue wq + first x tile first so qT matmuls can
        # start while everything else loads.
        wpool = ctx.enter_context(tc.tile_pool(name="wpool", bufs=1))
        wq_sb = [wpool.tile([P, DH], F16, tag=f"wq{s}", name=f"wq{s}") for s in range(8)]
        for s in range(8):
            nc.gpsimd.dma_start(out=wq_sb[s], in_=wq[s * P:(s + 1) * P, :])

        xT_v = xT.rearrange("(s p) m -> p s m", p=P)
        xtpool = ctx.enter_context(tc.tile_pool(name="xtpool", bufs=2))
        xt0 = xtpool.tile([P, 8, 512], F16, tag="xt", name="xt")
        nc.gpsimd.dma_start(out=xt0, in_=xT_v[:, :, 0:512])

        wk_sb = [wpool.tile([P, DH], F16, tag=f"wk{s}", name=f"wk{s}") for s in range(8)]
        wv_sb = [wpool.tile([P, DH], F16, tag=f"wv{s}", name=f"wv{s}") for s in range(8)]
        for s in range(8):
            nc.sync.dma_start(out=wk_sb[s], in_=wk[s * P:(s + 1) * P, :])
            nc.scalar.dma_start(out=wv_sb[s], in_=wv[s * P:(s + 1) * P, :])
        proj_pair = [const.tile([P, F], F16, tag=f"projp{i}", name=f"projp{i}")
                     for i in range(4)]
        for i in range(4):
            nc.sync.dma_start(out=proj_pair[i], in_=proj[i * P:(i + 1) * P, :])

        ones_sb = const.tile([P, P], F16, tag="ones_sb", name="ones_sb")
        nc.vector.memset(ones_sb, 1.0)

        # q_proj elu outputs for the first NQA groups, computed in pass A
        # (pass B is elementwise-bound; pass A is PE-bound -- shift work)
        NQA = 3
        qpa_pool = ctx.enter_context(tc.tile_pool(name="qpa", bufs=NQA))
        qPsaved = {}

        kvS = [[const.tile([P, P], F16, tag=f"kvS{h}_{s}", name=f"kvS{h}_{s}")
                for s in range(2)] for h in range(HPC)]
        ksr = [[const.tile([P, P], F16, tag=f"ksr{h}_{s}", name=f"ksr{h}_{s}")
                for s in range(2)] for h in range(HPC)]

        # ---------------- pass A ----------------
        with ExitStack() as actx:
            kvaccp = actx.enter_context(tc.tile_pool(name="kvaccp", bufs=1))
            kv_acc = [kvaccp.tile([P, 260], FP32, tag=f"kva{i}", name=f"kva{i}")
                      for i in range(4)]
            ktpool = actx.enter_context(tc.tile_pool(name="ktpool", bufs=2))
            qtpool = actx.enter_context(tc.tile_pool(name="qtpool", bufs=3))
            vpool = actx.enter_context(tc.tile_pool(name="vpool", bufs=2))
            ketmp = actx.enter_context(tc.tile_pool(name="ketmp", bufs=3))
            kppool = actx.enter_context(tc.tile_pool(name="kppool", bufs=2))
            mmps = actx.enter_context(tc.tile_pool(name="mmps", bufs=2, space="PSUM"))
            kpps = actx.enter_context(tc.tile_pool(name="kpps", bufs=5, space="PSUM"))
            kvps = actx.enter_context(tc.tile_pool(name="kvps", bufs=1, space="PSUM"))

            prev = None  # (kP tiles dict, vone) of group g-1

            def emit_kv_half(kv_ps, hp, j, pkP, pvone):
                """kv-state matmuls (head j of pair hp) of a PREVIOUS group."""
                for s in range(2):
                    co = j * 130 + s * 65
                    for t in range(TPG):
                        nc.tensor.matmul(
                            kv_ps[:, co:co + 65],
                            lhsT=(pkP[j][:, t * F + s * P: t * F + (s + 1) * P]),
                            rhs=(pvone[:, t, 2 * hp + j, :]),
                            start=(t == 0), stop=(t == TPG - 1),
                            skip_group_check=True)

            def fold_kv(hp, kv_ps, g):
                if g == 0:
                    nc.vector.tensor_copy(kv_acc[hp], kv_ps)
                else:
                    nc.vector.tensor_tensor(
                        out=kv_acc[hp], in0=kv_ps, in1=kv_acc[hp], op=AL.add)

            for g in range(NG):
                g0 = g * 512
                if g == 0:
                    xt = xt0
                else:
                    xt = xtpool.tile([P, 8, 512], F16, tag="xt", name="xt")
                    nc.gpsimd.dma_start(out=xt, in_=xT_v[:, :, g0:g0 + 512])

                # qT: spill to DRAM scratch (fp16)
                qt_g = []
                for fs in range(4):
                    ps = mmps.tile([P, 512], FP32, tag="mm", name="mm")
                    for s in range(8):
                        nc.tensor.matmul(
                            ps, lhsT=(wq_sb[s][:, fs * P:(fs + 1) * P]),
                            rhs=(xt[:, s, :]), start=(s == 0), stop=(s == 7))
                    qt_sb = qtpool.tile([P, 512], F16, tag="qt", name="qt")
                    nc.scalar.copy(qt_sb, ps)
                    qt_g.append(qt_sb)
                    if g >= NQA:
                        nc.gpsimd.dma_start(
                            out=qTd[fs * P:(fs + 1) * P, g0:g0 + 512], in_=qt_sb)

                # kT: kept in SBUF for this group
                kt_sb = [ktpool.tile([P, 512], F16, tag=f"kt{fs}", name=f"kt{fs}")
                         for fs in range(4)]
                for fs in range(4):
                    ps = mmps.tile([P, 512], FP32, tag="mm", name="mm")
                    for s in range(8):
                        nc.tensor.matmul(
                            ps, lhsT=(wk_sb[s][:, fs * P:(fs + 1) * P]),
                            rhs=(xt[:, s, :]), start=(s == 0), stop=(s == 7))
                    nc.scalar.copy(kt_sb[fs], ps)

                # v with ones column: vone[p, t, h, 0:64]=v, [..,64]=1
                # (pool depth 2 -> generation g reuses g-2's buffer, whose
                # ones column is already set; write it only for g<2)
                vone = vpool.tile([P, TPG, HPC, HD + 1], F16, tag="vone", name="vone")
                if g < 2:
                    nc.scalar.copy(
                        vone[:, :, :, HD],
                        ones_sb[:, 0:TPG * HPC].rearrange(
                            "p (t h) -> p t h", t=TPG))
                for t in range(TPG):
                    ps = mmps.tile([P, 512], FP32, tag="mm", name="mm")
                    for s in range(8):
                        nc.tensor.matmul(
                            ps, lhsT=(xt[:, s, t * P:(t + 1) * P]),
                            rhs=(wv_sb[s]), start=(s == 0), stop=(s == 7))
                    nc.vector.tensor_copy(
                        vone[:, t, :, 0:HD],
                        ps.rearrange("p (h e) -> p h e", h=HPC))

                # k_proj + elu1 for THIS group; kv matmuls for the PREVIOUS
                # group interleaved per pair to keep the PE streaming while
                # this group's elu chain runs on Scalar/DVE/GpSimd.
                kP_g = []
                for hp in range(HPC // 2):
                    kE = [ketmp.tile([P, 1024], F16, tag=f"kE{j}", name=f"kE{j}")
                          for j in range(2)]
                    kR = [ketmp.tile([P, 1024], F16, tag=f"kR{j}", name=f"kR{j}")
                          for j in range(2)]
                    kP = [kppool.tile([P, 1024], F16, tag=f"kP{hp}_{j}",
                                      name=f"kP{hp}_{j}") for j in range(2)]
                    kv_ps = kvps.tile([P, 260], FP32, tag="kvg", name="kvg") \
                        if prev is not None else None
                    for tp in range(2):
                        kps = []
                        for j in range(2):
                            hb = j * HD
                            kp = kpps.tile([P, 512], FP32, tag="kp", name="kp")
                            for ti in range(2):
                                t = tp * 2 + ti
                                nc.tensor.matmul(
                                    kp[:, ti * F:(ti + 1) * F],
                                    lhsT=(kt_sb[hp][hb:hb + HD,
                                                    t * P:(t + 1) * P]),
                                    rhs=(proj_pair[hp][hb:hb + HD, :]),
                                    start=True, stop=True,
                                    skip_group_check=True)
                            kps.append(kp)
                        for j in range(2):
                            nc.scalar.activation(
                                kE[j][:, tp * 512:(tp + 1) * 512], kps[j], AF.Exp)
                            nc.vector.tensor_scalar(
                                kR[j][:, tp * 512:(tp + 1) * 512], kps[j],
                                0.0, 1.0, op0=AL.max, op1=AL.add)
                        # kv matmuls of the PREVIOUS group for head j=tp,
                        # between this group's kproj bursts: spreads PSUM
                        # bank reuse and keeps the PE streaming.
                        if kv_ps is not None:
                            emit_kv_half(kv_ps, hp, tp, prev[0][hp], prev[1])
                    for j in range(2):
                        nc.vector.tensor_tensor(
                            out=kP[j], in0=kE[j], in1=kR[j], op=AL.min)
                    kP_g.append(kP)
                    if kv_ps is not None:
                        fold_kv(hp, kv_ps, g - 1)
                    if g < NQA:
                        # q_proj + elu1 for this (early) group's pair, using
                        # the un-spilled qT tiles; results held in SBUF for
                        # pass B.  PSUM rides the kproj pool rotation.
                        qE2 = [ketmp.tile([P, 1024], F16, tag=f"qeA{j}",
                                          name=f"qeA{j}") for j in range(2)]
                        qR2 = [ketmp.tile([P, 1024], F16, tag=f"qrA{j}",
                                          name=f"qrA{j}") for j in range(2)]
                        qP2 = [qpa_pool.tile([P, 1024], F16, tag=f"qPa{hp}_{j}",
                                             name=f"qPa{hp}_{j}")
                               for j in range(2)]
                        for s in range(2):
                            qps2 = []
                            for j in range(2):
                                hb = j * HD
                                ps = kpps.tile([P, 512], FP32, tag="kp",
                                               name="qpa_ps")
                                nc.tensor.matmul(
                                    ps, lhsT=(proj_pair[hp][hb:hb + HD,
                                                            s * P:(s + 1) * P]),
                                    rhs=(qt_g[hp][hb:hb + HD, :]),
                                    start=True, stop=True)
                                qps2.append(ps)
                            for j in range(2):
                                nc.scalar.activation(
                                    qE2[j][:, s * 512:(s + 1) * 512], qps2[j],
                                    AF.Exp)
                                nc.vector.tensor_scalar(
                                    qR2[j][:, s * 512:(s + 1) * 512], qps2[j],
                                    0.0, 1.0, op0=AL.max, op1=AL.add)
                        for j in range(2):
                            nc.vector.tensor_tensor(
                                out=qP2[j], in0=qE2[j], in1=qR2[j], op=AL.min)
                        qPsaved[(g, hp)] = qP2
                prev = (kP_g, vone)

            for hp in range(HPC // 2):
                kv_ps = kvps.tile([P, 260], FP32, tag="kvg", name="kvg")
                for j in range(2):
                    emit_kv_half(kv_ps, hp, j, prev[0][hp], prev[1])
                fold_kv(hp, kv_ps, NG - 1)

            # zero the attn lhsT pads late so the memsets don't clog the
            # DVE queue ahead of group-0's PSUM readers
            for h in range(HPC):
                for s in range(2):
                    nc.vector.memset(kvS[h][s], 0.0)
                    nc.vector.memset(ksr[h][s], 0.0)

            # ------- kv fixup: kv_acc -> kvS (cast, GpSimd) / ksr (Scalar) ----
            for hp in range(4):
                for j in range(2):
                    h = 2 * hp + j
                    hb = j * HD
                    for s in range(2):
                        co = j * 130 + s * 65
                        nc.vector.tensor_copy(
                            kvS[h][s][:, hb:hb + HD],
                            kv_acc[hp][:, co:co + HD])
                        nc.scalar.activation(
                            ksr[h][s][:, hb:hb + HD], ones_sb[:, 0:HD],
                            AF.Copy, scale=kv_acc[hp][:, co + HD:co + HD + 1])

        # ---------------- pass B ----------------
        with ExitStack() as bctx:
            wopool = bctx.enter_context(tc.tile_pool(name="wopool", bufs=1))
            wo_sb = [wopool.tile([P, D], F16, tag=f"wo{s}", name=f"wo{s}") for s in range(4)]
            for s in range(4):
                nc.gpsimd.dma_start(out=wo_sb[s], in_=wout[s * P:(s + 1) * P, :])

            qtbpool = bctx.enter_context(tc.tile_pool(name="qtbpool", bufs=2))
            qepool = bctx.enter_context(tc.tile_pool(name="qepool", bufs=3))
            qppool = bctx.enter_context(tc.tile_pool(name="qppool", bufs=2))
            attpool = bctx.enter_context(tc.tile_pool(name="attpool", bufs=2))
            zpool = bctx.enter_context(tc.tile_pool(name="zpool", bufs=2))
            ypool = bctx.enter_context(tc.tile_pool(name="ypool", bufs=3))
            qpps = bctx.enter_context(tc.tile_pool(name="qpps", bufs=4, space="PSUM"))
            atps = bctx.enter_context(tc.tile_pool(name="atps", bufs=2, space="PSUM"))
            dnps = bctx.enter_context(tc.tile_pool(name="dnps", bufs=1, space="PSUM"))
            yps = bctx.enter_context(tc.tile_pool(name="yps", bufs=1, space="PSUM"))

            qTd_v = qTd.rearrange("(hh p) m -> p hh m", p=P)

            att_prev = None
            g0_prev = 0

            def emit_y_half(t, o, patt, pg0):
                """y matmuls for token tile t, output half o, of a
                PREVIOUS group.  The eviction is emitted before the current
                slot's exps so it sits ahead of them in the scalar queue."""
                pso = yps.tile([P, 512], FP32, tag="y", name="pso")
                for s in range(4):
                    nc.tensor.matmul(
                        pso, lhsT=(patt[s][:, t * P:(t + 1) * P]),
                        rhs=(wo_sb[s][:, o * 512:(o + 1) * 512]),
                        start=(s == 0), stop=(s == 3))
                y_sb = ypool.tile([P, 512], F16, tag="ysb", name="ysb")
                nc.scalar.copy(y_sb, pso)
                nc.gpsimd.dma_start(
                    out=y[pg0 + t * P: pg0 + (t + 1) * P,
                          o * 512:(o + 1) * 512],
                    in_=y_sb)

            for g in range(NG):
                g0 = g * 512
                early = g < NQA
                if not early:
                    qt = qtbpool.tile([P, HPC // 2, 512], F16, tag="qt", name="qt")
                    nc.gpsimd.dma_start(out=qt, in_=qTd_v[:, :, g0:g0 + 512])

                # q_proj matmuls + elu1 for this group (early groups: done in
                # pass A), y matmuls for the previous group interleaved.
                qP_g = []
                for hp in range(HPC // 2):
                    if early:
                        qP_g.append(qPsaved[(g, hp)])
                        if att_prev is not None:
                            emit_y_half(hp, 0, att_prev, g0_prev)
                            emit_y_half(hp, 1, att_prev, g0_prev)
                        continue
                    qE = [qepool.tile([P, 1024], F16, tag=f"qE{j}", name=f"qE{j}")
                          for j in range(2)]
                    qR = [qepool.tile([P, 1024], F16, tag=f"qR{j}", name=f"qR{j}")
                          for j in range(2)]
                    qP = [qppool.tile([P, 1024], F16, tag=f"qP{hp}_{j}",
                                      name=f"qP{hp}_{j}") for j in range(2)]
                    for s in range(2):
                        qps = []
                        for j in range(2):
                            hb = j * HD
                            ps = qpps.tile([P, 512], FP32, tag="qp", name="qp")
                            nc.tensor.matmul(
                                ps, lhsT=(proj_pair[hp][hb:hb + HD,
                                                        s * P:(s + 1) * P]),
                                rhs=(qt[hb:hb + HD, hp, :]),
                                start=True, stop=True)
                            qps.append(ps)
                        if att_prev is not None:
                            emit_y_half(hp, s, att_prev, g0_prev)
                        for j in range(2):
                            nc.scalar.activation(
                                qE[j][:, s * 512:(s + 1) * 512], qps[j], AF.Exp)
                            nc.vector.tensor_scalar(
                                qR[j][:, s * 512:(s + 1) * 512], qps[j],
                                0.0, 1.0, op0=AL.max, op1=AL.add)
                    for j in range(2):
                        nc.vector.tensor_tensor(
                            out=qP[j], in0=qE[j], in1=qR[j], op=AL.min)
                    qP_g.append(qP)

                att_sb = [attpool.tile([P, 512], F16, tag=f"att{i}", name=f"att{i}")
                          for i in range(4)]
                for hp in range(HPC // 2):
                    aps = atps.tile([P, 512], FP32, tag="at", name="aps")
                    dps = dnps.tile([P, 512], FP32, tag="dn", name="dps")
                    for j in range(2):
                        h = 2 * hp + j
                        for s in range(2):
                            first = j == 0 and s == 0
                            last = j == 1 and s == 1
                            rhs = qP_g[hp][j][:, s * 512:(s + 1) * 512]
                            nc.tensor.matmul(
                                aps, lhsT=(kvS[h][s]), rhs=rhs,
                                start=first, stop=last, skip_group_check=True)
                            nc.tensor.matmul(
                                dps, lhsT=(ksr[h][s]), rhs=rhs,
                                start=first, stop=last, skip_group_check=True)
                    zb = zpool.tile([P, 512], FP32, tag="zb", name="zb")
                    nc.vector.reciprocal_approx_fast(zb, dps)
                    nc.vector.tensor_tensor(
                        out=att_sb[hp], in0=aps, in1=zb, op=AL.mult)

                att_prev = att_sb
                g0_prev = g0

            for t in range(TPG):
                for o in range(2):
                    emit_y_half(t, o, att_prev, g0_prev)


def build(n=SEQ):
    nc = bacc.Bacc("TRN2", target_bir_lowering=False, debug=False,
                   enable_asserts=False)
    xT = nc.declare_dram_parameter("xT", [D, n], F16, isOutput=False)
    wq = nc.declare_dram_parameter("wq", [D, DH], F16, isOutput=False)
    wk = nc.declare_dram_parameter("wk", [D, DH], F16, isOutput=False)
    wv = nc.declare_dram_parameter("wv", [D, DH], F16, isOutput=False)
    proj = nc.declare_dram_parameter("proj", [DH, F], F16, isOutput=False)
    wout = nc.declare_dram_parameter("wout", [DH, D], F16, isOutput=False)
    y = nc.declare_dram_parameter("y", [n, D], F16, isOutput=True)
    qTd = nc.dram_tensor("qT_scratch", [DH, n], F16)
    with tile.TileContext(nc) as tc:
        _emit(tc, n, xT, wq, wk, wv, proj, wout, y, qTd)
    nc.finalize()
    return nc


def make_in_maps(x, w_qkv, proj_matrix, w_out):
    x = np.asarray(x, np.float32)
    w_qkv = np.asarray(w_qkv, np.float32)
    proj_matrix = np.asarray(proj_matrix, np.float32)
    w_out = np.asarray(w_out, np.float32)
    in_maps = []
    for c in range(NCORES):
        b, g = c // 2, c % 2
        in_maps.append({
            "xT": x[b].T.astype(np.float16),
            "wq": w_qkv[:, DH * g:DH * (g + 1)].astype(np.float16),
            "wk": w_qkv[:, D + DH * g:D + DH * (g + 1)].astype(np.float16),
            "wv": w_qkv[:, 2 * D + DH * g:2 * D + DH * (g + 1)].astype(np.float16),
            "proj": proj_matrix[HPC * g:HPC * (g + 1)].reshape(DH, F)
                    .astype(np.float16),
            "wout": w_out[DH * g:DH * (g + 1), :].astype(np.float16),
        })
    return in_maps


_NC_CACHE = {}


def get_nc(n=SEQ):
    if n not in _NC_CACHE:
        _NC_CACHE[n] = build(n)
    return _NC_CACHE[n]


def _install_ntff_hook_shim():
    """The agent image's antenv lacks axon_hooks; recreate it so
    run_bass_kernel_spmd(trace=True) can capture NTFF profiles."""
    import sys
    import types
    try:
        from antenv.axon_hooks import get_axon_ntff_profile_hook  # noqa: F401
        return True
    except ImportError:
        pass
    try:
        from trn_agent_boot.trn_boot import _ntff_profile_via_ctypes
        import antenv
        mod = types.ModuleType("antenv.axon_hooks")
        mod._hook = _ntff_profile_via_ctypes("/opt/axon/libaxon_pjrt.so")
        mod.set_axon_ntff_profile_hook = lambda h: setattr(mod, "_hook", h)
        mod.get_axon_ntff_profile_hook = lambda: mod._hook
        sys.modules["antenv.axon_hooks"] = mod
        antenv.axon_hooks = mod
        return True
    except Exception as e:  # profiling is best-effort
        print(f"ntff hook shim failed: {e}")
        return False


def run(x, w_qkv, proj_matrix, w_out, b_out, trace=False, **kw):
    if trace:
        _install_ntff_hook_shim()
    nc = get_nc(SEQ)
    in_maps = make_in_maps(x, w_qkv, proj_matrix, w_out)
    res = run_bass_kernel_spmd(nc, in_maps, list(range(NCORES)),
                               trace=trace, **kw)
    b_out = np.asarray(b_out, np.float32)
    out = np.empty((B, SEQ, D), np.float32)
    for b in range(B):
        out[b] = res.results[2 * b]["y"].astype(np.float32) \
            + res.results[2 * b + 1]["y"].astype(np.float32) \
            + b_out[None, :]
    return out, res


def kernel(x, w_qkv, proj_matrix, w_out, b_out):
    out, _ = run(x, w_qkv, proj_matrix, w_out, b_out)
    return out
